# revision 1
# baseline (speedup 1.0000x reference)
"""CrossAttentionFusion Bass kernel for 8 TRN2 NeuronCores.

Reference computation (T=4096, B=64, D=64):
    q = eeg @ Wq.T + bq ; k = fnirs @ Wk.T + bk ; v = fnirs @ Wv.T + bv
    score = sum(q*k, -1) * D**-0.5        # [T, B, 1]
    attn = softmax(score, axis=0)         # over T
    out = eeg + attn * v

Strategy highlights:
  - Data-parallel over batch: core c handles batches [8c, 8c+8).
  - Algebraic fold: score*SCALE = x^T G y + w.x + u.y (+const dropped),
    G = SCALE*Wq^T@Wk, w = SCALE*Wq^T@bk, u = SCALE*Wk^T@bq — removes the
    q/k projections.
  - [feature, token] layout on device, host packs bf16 [128, 2048]
    superchunk tiles ([x; y] stacked); fp32 PSUM accumulation.
  - ALL matmuls use K=128: the PE HAM throttle keeps half-height (K=64)
    matmuls at 1.2 GHz forever; full-height streams run at 2.4 GHz.
      * one block-diagonal [[G,0],[0,Wv^T]] matmul -> z and v
      * one reduce-matmul with rhs [x; m] (m = (z+u)*y) and one-hot
        lhsT column [w; ones] -> per-(batch,chunk) score row; all 32
        (batch, chunk) rows of a superchunk accumulate into ONE PSUM
        bank [32, 512]
      * pass-B broadcast matmul: one-hot row lhsT against a zero-padded
        [128, *] attn tile
  - Softmax runs on [32, 512] tiles (4 chunks x 8 batches on partitions):
    2 exp ops total; per-batch max/sum merges use tiny SBUF-SBUF
    partition-remap DMAs.
  - Pass B: DVE multiplies broadcast attn by v, DVE/GpSimd add the fp32
    eeg residual (prefetched into SBUF during pass A), DMA out.
"""

import sys

sys.path.insert(0, "/opt/trn_rl_repo")

import ml_dtypes
import numpy as np

import concourse.bass as bass
import concourse.tile as tile
from concourse import bacc, mybir

T, B, D = 4096, 64, 64
N_CORES = 8
BC = B // N_CORES  # 8 batches per core
NCH = 8  # 512-token chunks along T
CH = T // NCH  # 512
NSC = 2  # superchunks (DMA + softmax granularity)
SCW = T // NSC // CH  # 4 chunks per superchunk
NPAIR = NCH // 2  # chunk pairs (128-partition packing in pass B)
SCALE = float(D) ** -0.5
F32 = mybir.dt.float32
BF16 = mybir.dt.bfloat16
NPBF16 = ml_dtypes.bfloat16
AF = mybir.ActivationFunctionType
ALU = mybir.AluOpType

_CACHE = {}


def _build_nc():
    nc = bacc.Bacc(
        "TRN2", target_bir_lowering=False, debug=False, num_devices=N_CORES
    )

    xy_d = nc.dram_tensor(
        "XY", [NSC, BC, 128, SCW * CH], BF16, kind="ExternalInput"
    ).ap()
    eegr_d = nc.dram_tensor(
        "EEGR", [BC, 128, NPAIR * CH], F32, kind="ExternalInput"
    ).ap()
    big_d = nc.dram_tensor("BIG", [128, 128], BF16, kind="ExternalInput").ap()
    wones_d = nc.dram_tensor(
        "WONES", [128, BC * SCW * 32], BF16, kind="ExternalInput"
    ).ap()
    u_d = nc.dram_tensor("UVEC", [128, 1], F32, kind="ExternalInput").ap()
    bv_d = nc.dram_tensor("BV2", [128, 1], F32, kind="ExternalInput").ap()
    oh_d = nc.dram_tensor(
        "OH32", [128, 32 * 64], BF16, kind="ExternalInput"
    ).ap()
    out_d = nc.dram_tensor(
        "OUT", [BC, NPAIR, 128, CH], F32, kind="ExternalOutput"
    ).ap()

    with tile.TileContext(nc) as tc:
        with (
            tc.tile_pool(name="consts", bufs=1) as consts,
            tc.tile_pool(name="store", bufs=1) as store,
            tc.tile_pool(name="xy", bufs=2) as xyp,
            tc.tile_pool(name="mstack", bufs=2) as msp,
            tc.tile_pool(name="sm", bufs=1) as smp,
            tc.tile_pool(name="passb", bufs=4) as pbp,
            tc.tile_pool(name="pzv", bufs=5, space="PSUM") as pzvp,
            tc.tile_pool(name="psc", bufs=2, space="PSUM") as pscp,
            tc.tile_pool(name="psmall", bufs=1, space="PSUM") as psmp,
        ):
            big_s = consts.tile([128, 128], BF16)
            nc.sync.dma_start(big_s[:], big_d[:])
            wones_s = consts.tile([128, BC * SCW * 32], BF16)
            nc.sync.dma_start(wones_s[:], wones_d[:])
            u_s = consts.tile([128, 1], F32)
            nc.sync.dma_start(u_s[:], u_d[:])
            bv_s = consts.tile([128, 1], F32)
            nc.sync.dma_start(bv_s[:], bv_d[:])
            oh_s = consts.tile([128, 32 * 64], BF16)
            nc.sync.dma_start(oh_s[:], oh_d[:])

            # persistent stores: eeg (fp32 residual) and v, [128, 512] x 32
            xstore = store.tile([128, NPAIR * BC * CH], F32)  # 64 KB/part
            vstore = store.tile([128, NPAIR * BC * CH], F32)  # 64 KB/part

            # attn, padded to 128 partitions (rows 32: stay zero)
            attn_s = smp.tile([128, NSC * CH], BF16)
            nc.gpsimd.memset(attn_s[:], 0.0)

            def stsl(bi, pi):
                return slice((bi * NPAIR + pi) * CH, (bi * NPAIR + pi + 1) * CH)

            # ---------------- pass A ----------------
            scr32s = []
            for sc in range(NSC):
                pscall = pscp.tile([32, CH], F32, tag="psc", name=f"psc{sc}")
                for b in range(BC):
                    xy = xyp.tile([128, SCW * CH], BF16, tag="xy")
                    nc.sync.dma_start(xy[:], xy_d[sc, b])
                    if sc == 0:
                        # prefetch fp32 eeg residual for pass B
                        nc.sync.dma_start(
                            xstore[:, b * NPAIR * CH : (b + 1) * NPAIR * CH],
                            eegr_d[b],
                        )
                    ms = msp.tile([128, SCW * CH], BF16, tag="ms")
                    # rows 0:64 of the reduce rhs = x (SBUF->SBUF DMA copy)
                    nc.sync.dma_start(ms[0:64, :], xy[0:64, :])
                    for cq in range(SCW):
                        c = sc * SCW + cq
                        csl = slice(cq * CH, (cq + 1) * CH)
                        pzv = pzvp.tile([128, CH], F32, tag="pzv")
                        nc.tensor.matmul(
                            pzv[:], big_s[:], xy[:, csl], start=True, stop=True
                        )
                        half = slice((c % 2) * 64, (c % 2) * 64 + 64)
                        # v = Wv@y + bv, keep for pass B
                        nc.scalar.activation(
                            vstore[half, stsl(b, c // 2)],
                            pzv[64:128, :],
                            AF.Identity,
                            bias=bv_s[half, :],
                        )
                        # m = (z + u) * y -> rows 64:128 of the reduce rhs
                        # (DVE only: GpSimd cannot read PSUM)
                        nc.vector.scalar_tensor_tensor(
                            ms[64:128, csl],
                            pzv[0:64, :],
                            u_s[64:128, :],
                            xy[64:128, csl],
                            op0=ALU.add,
                            op1=ALU.mult,
                        )
                        # scores row 8*cq+b += w.x + sum_d m   (K=128)
                        wsl = slice((b * SCW + cq) * 32, (b * SCW + cq + 1) * 32)
                        nc.tensor.matmul(
                            pscall[:],
                            wones_s[:, wsl],
                            ms[:, csl],
                            start=(b == 0 and cq == 0),
                            stop=(b == BC - 1 and cq == SCW - 1),
                        )
                scr32 = smp.tile([32, CH], F32, name=f"scr32_{sc}", tag=f"scr32_{sc}")
                nc.scalar.activation(scr32[:], pscall[:], AF.Identity, bias=0.0)
                scr32s.append(scr32)

            # ---------------- softmax over T ----------------
            # row layout of scr32s[sc]: 8*cq + b holds chunk sc*SCW+cq, batch b
            mxp = smp.tile([32, NSC], F32)
            for sc in range(NSC):
                nc.vector.tensor_reduce(
                    mxp[:, sc : sc + 1],
                    scr32s[sc][:],
                    axis=mybir.AxisListType.X,
                    op=ALU.max,
                )
            pmax = smp.tile([32, 1], F32)
            nc.vector.tensor_reduce(
                pmax[:], mxp[:], axis=mybir.AxisListType.X, op=ALU.max
            )
            pm84 = smp.tile([BC, SCW], F32)
            for cq in range(SCW):
                nc.sync.dma_start(
                    pm84[:, cq : cq + 1], pmax[8 * cq : 8 * cq + 8, :]
                )
            mx = smp.tile([BC, 1], F32)
            nc.vector.tensor_reduce(
                mx[:], pm84[:], axis=mybir.AxisListType.X, op=ALU.max
            )
            nmx = smp.tile([BC, 1], F32)
            nc.vector.tensor_scalar_mul(nmx[:], mx[:], -1.0)
            nmx32 = smp.tile([32, 1], F32)
            for cq in range(SCW):
                nc.sync.dma_start(nmx32[8 * cq : 8 * cq + 8, :], nmx[:])
            zs32 = smp.tile([32, NSC], F32)
            for sc in range(NSC):
                nc.scalar.activation(
                    attn_s[0:32, sc * CH : (sc + 1) * CH],
                    scr32s[sc][:],
                    AF.Exp,
                    bias=nmx32[:],
                    accum_out=zs32[:, sc : sc + 1],
                )
            zsA = smp.tile([32, 1], F32)
            nc.vector.tensor_reduce(
                zsA[:], zs32[:], axis=mybir.AxisListType.X, op=ALU.add
            )
            zs84 = smp.tile([BC, SCW], F32)
            for cq in range(SCW):
                nc.sync.dma_start(
                    zs84[:, cq : cq + 1], zsA[8 * cq : 8 * cq + 8, :]
                )
            zsum = smp.tile([BC, 1], F32)
            nc.vector.tensor_reduce(
                zsum[:], zs84[:], axis=mybir.AxisListType.X, op=ALU.add
            )
            rz = smp.tile([BC, 1], F32)
            nc.vector.reciprocal(rz[:], zsum[:])
            rz32 = smp.tile([32, 1], F32)
            for cq in range(SCW):
                nc.sync.dma_start(rz32[8 * cq : 8 * cq + 8, :], rz[:])
            for sc in range(NSC):
                ssl = slice(sc * CH, (sc + 1) * CH)
                nc.vector.tensor_scalar_mul(
                    attn_s[0:32, ssl], attn_s[0:32, ssl], rz32[:]
                )

            # ---------------- pass B ----------------
            for b in range(BC):
                for p in range(NPAIR):
                    pa2 = pzvp.tile([128, CH], F32, tag="pzv", name=f"pa2_{b}_{p}")
                    for h in range(2):
                        c = 2 * p + h
                        sc, cq = divmod(c, SCW)
                        osl = slice((8 * cq + b) * 64, (8 * cq + b + 1) * 64)
                        nc.tensor.matmul(
                            pa2[h * 64 : h * 64 + 64, :],
                            oh_s[:, osl],
                            attn_s[:, sc * CH : (sc + 1) * CH],
                            start=True,
                            stop=True,
                        )
                    tav = pbp.tile([128, CH], F32, tag="tav")
                    nc.vector.tensor_mul(tav[:], pa2[:], vstore[:, stsl(b, p)])
                    o2 = pbp.tile([128, CH], F32, tag="o2")
                    eng = nc.vector if (p % 2 == 0) else nc.gpsimd
                    eng.tensor_add(o2[:], tav[:], xstore[:, stsl(b, p)])
                    nc.sync.dma_start(out_d[b, p], o2[:])

    nc.compile()
    return nc


def _get_nc():
    if "nc" not in _CACHE:
        _CACHE["nc"] = _build_nc()
    return _CACHE["nc"]


def _host_constants(Wq, bq, Wk, bk, Wv, bv):
    Wq64, Wk64, Wv64 = (np.asarray(a, np.float64) for a in (Wq, Wk, Wv))
    bq64, bk64 = np.asarray(bq, np.float64), np.asarray(bk, np.float64)
    G = SCALE * (Wq64.T @ Wk64)  # [d, e]
    w = SCALE * (Wq64.T @ bk64)  # [64]
    u = SCALE * (Wk64.T @ bq64)  # [64]

    BIG = np.zeros((128, 128), np.float64)
    BIG[0:64, 0:64] = G  # z[e,n] = sum_d G[d,e] x[d,n]
    BIG[64:128, 64:128] = Wv64.T  # v[o,n] = sum_e Wv[o,e] y[e,n]

    # reduce-matmul lhsT blocks [128, 32] per (b, cq): single non-zero
    # column 8*cq+b = [w (x rows); ones (m rows)]
    WONES = np.zeros((128, BC * SCW * 32), np.float32)
    for b in range(BC):
        for cq in range(SCW):
            col = (b * SCW + cq) * 32 + 8 * cq + b
            WONES[0:64, col] = w
            WONES[64:128, col] = 1.0

    U = np.tile(u.reshape(64, 1), (2, 1)).astype(np.float32)
    BV2 = np.tile(np.asarray(bv, np.float32).reshape(64, 1), (2, 1))

    # pass-B broadcast lhsT: block per (8*cq+b) with one-hot row
    OH32 = np.zeros((128, 32 * 64), np.float32)
    for r in range(32):
        OH32[r, r * 64 : (r + 1) * 64] = 1.0
    return (
        BIG.astype(NPBF16),
        WONES.astype(NPBF16),
        U,
        BV2,
        OH32.astype(NPBF16),
    )


def _pack_inputs(eeg, fnirs):
    # [T, B, D] -> XY[core, sc, b, feat, SCW*CH]; tok index = cq*CH + t
    def tr(x):
        x = np.asarray(x, np.float32).reshape(NSC, SCW, CH, N_CORES, BC, D)
        x = x.transpose(3, 0, 4, 5, 1, 2)  # [core, sc, b, d, cq, t]
        return x.reshape(N_CORES, NSC, BC, D, SCW * CH)

    XY = np.empty((N_CORES, NSC, BC, 128, SCW * CH), NPBF16)
    XY[:, :, :, 0:64, :] = tr(eeg)
    XY[:, :, :, 64:128, :] = tr(fnirs)
    # fp32 eeg residual, pass-B layout: [core, b, half*64+d, p*CH+t]
    e = np.asarray(eeg, np.float32).reshape(NPAIR, 2, CH, N_CORES, BC, D)
    e = e.transpose(3, 4, 1, 5, 0, 2)  # [core, b, half, d, pair, tok]
    EEGR = np.ascontiguousarray(e).reshape(N_CORES, BC, 128, NPAIR * CH)
    return XY, EEGR


def _prepare(eeg, fnirs, Wq, bq, Wk, bk, Wv, bv):
    BIG, WONES, U, BV2, OH32 = _host_constants(Wq, bq, Wk, bk, Wv, bv)
    XY, EEGR = _pack_inputs(eeg, fnirs)
    return [
        {
            "XY": np.ascontiguousarray(XY[c]),
            "EEGR": EEGR[c],
            "BIG": BIG,
            "WONES": WONES,
            "UVEC": U,
            "BV2": BV2,
            "OH32": OH32,
        }
        for c in range(N_CORES)
    ]


def _unpack_output(outs):
    # outs: list of [BC, NPAIR, 128, CH] per core -> [T, B, D]
    o = np.stack(outs)  # [core, b, pair, 128, tok]
    o = o.reshape(N_CORES, BC, NPAIR, 2, D, CH)  # [core, b, pair, half, d, tok]
    o = o.transpose(2, 3, 5, 0, 1, 4)  # [pair, half, tok, core, b, d]
    return np.ascontiguousarray(o.reshape(T, B, D))


def _run(eeg, fnirs, Wq, bq, Wk, bk, Wv, bv, **spmd_kwargs):
    from concourse.bass_utils import run_bass_kernel_spmd

    nc = _get_nc()
    in_maps = _prepare(eeg, fnirs, Wq, bq, Wk, bk, Wv, bv)
    res = run_bass_kernel_spmd(nc, in_maps, list(range(N_CORES)), **spmd_kwargs)
    return _unpack_output([res.results[c]["OUT"] for c in range(N_CORES)]), res


def kernel(eeg, fnirs, Wq, bq, Wk, bk, Wv, bv):
    return _run(eeg, fnirs, Wq, bq, Wk, bk, Wv, bv)[0]



# revision 2
# speedup vs baseline: 1.6749x; 1.6749x over previous
"""CrossAttentionFusion Bass kernel for 8 TRN2 NeuronCores.

Reference computation (T=4096, B=64, D=64):
    q = eeg @ Wq.T + bq ; k = fnirs @ Wk.T + bk ; v = fnirs @ Wv.T + bv
    score = sum(q*k, -1) * D**-0.5        # [T, B, 1]
    attn = softmax(score, axis=0)         # over T
    out = eeg + attn * v

Design (v2):
  - Data-parallel over batch: core c handles batches [8c, 8c+8).
  - Algebraic fold: score = x^T G y + w.x + u.y (+const dropped),
    G = SCALE*Wq^T@Wk, w = SCALE*Wq^T@bk, u = SCALE*Wk^T@bq.
  - Pair-tile layout [chunk 2p feats; chunk 2p+1 feats] x 512 tokens, bf16.
    eeg tiles stay resident in SBUF and serve as the pass-B residual
    (no second fp32 eeg load); output is written bf16 (host upcasts).
    HBM traffic/core: 8 MB in + 4 MB out (vs 24 MB in v1).
  - Pass A per (pair, batch): two block-diag matmuls (z = G^T x, v = Wv y),
    one full-height DVE op m = (z+u)*y, two K=128 reduce matmuls
    accumulating w.x + sum(m) into a [16, 512] score bank per pair
    (rows = 2*batch + chunk-parity).
  - Softmax merges across partitions use the DVE 32x32 block transpose
    (no SBUF->SBUF DMA remaps). Exp runs on the scalar engine with
    per-row bias = -max_b; 1/Z is folded into pass B's
    scalar_tensor_tensor for free (the [P,1] scalar slot), so attn is
    never normalized in place.
  - Pass B per (batch, pair): one K=16 broadcast matmul replicates the
    attn row pair into [128, 512]; DVE computes (attn*rz_b)*v in one
    scalar_tensor_tensor; residual add runs bf16 (2x DVE mode) split
    with GpSimd; output DMA'd bf16.
"""

import sys

sys.path.insert(0, "/opt/trn_rl_repo")

import ml_dtypes
import numpy as np

import concourse.bass as bass
import concourse.tile as tile
from concourse import bacc, mybir

T, B, D = 4096, 64, 64
N_CORES = 8
BC = B // N_CORES  # 8 batches per core
CH = 512  # tokens per chunk
NCH = T // CH  # 8 chunks
NP = NCH // 2  # 4 chunk pairs
SCALE = float(D) ** -0.5
F32 = mybir.dt.float32
BF16 = mybir.dt.bfloat16
NPBF16 = ml_dtypes.bfloat16
AF = mybir.ActivationFunctionType
ALU = mybir.AluOpType

_CACHE = {}


def _build_nc():
    nc = bacc.Bacc(
        "TRN2", target_bir_lowering=False, debug=False, num_devices=N_CORES
    )

    xt_d = nc.dram_tensor("XT", [BC, NP, 128, CH], BF16, kind="ExternalInput").ap()
    yt_d = nc.dram_tensor("YT", [BC, NP, 128, CH], BF16, kind="ExternalInput").ap()
    bigg_d = nc.dram_tensor("BIGG", [128, 128], BF16, kind="ExternalInput").ap()
    bigw_d = nc.dram_tensor("BIGW", [128, 128], BF16, kind="ExternalInput").ap()
    wcols_d = nc.dram_tensor("WCOLS", [128, BC * 16], BF16, kind="ExternalInput").ap()
    onesc_d = nc.dram_tensor("ONESC", [128, BC * 16], BF16, kind="ExternalInput").ap()
    oh8_d = nc.dram_tensor("OH8", [16, BC * 128], BF16, kind="ExternalInput").ap()
    u2_d = nc.dram_tensor("U2", [128, 1], F32, kind="ExternalInput").ap()
    bv2_d = nc.dram_tensor("BV2", [128, 1], F32, kind="ExternalInput").ap()
    one1_d = nc.dram_tensor("ONE1", [1, 128], F32, kind="ExternalInput").ap()
    out_d = nc.dram_tensor("OUT", [BC, NP, 128, CH], BF16, kind="ExternalOutput").ap()

    with tile.TileContext(nc) as tc:
        with (
            tc.tile_pool(name="consts", bufs=1) as consts,
            tc.tile_pool(name="store", bufs=1) as store,
            tc.tile_pool(name="yt", bufs=3) as ytp,
            tc.tile_pool(name="ms", bufs=3) as msp,
            tc.tile_pool(name="sm", bufs=1) as smp,
            tc.tile_pool(name="passb", bufs=6) as pbp,
            tc.tile_pool(name="pzv", bufs=4, space="PSUM") as pzvp,
            tc.tile_pool(name="psc", bufs=4, space="PSUM") as pscp,
        ):
            bigg_s = consts.tile([128, 128], BF16)
            nc.sync.dma_start(bigg_s[:], bigg_d[:])
            bigw_s = consts.tile([128, 128], BF16)
            nc.sync.dma_start(bigw_s[:], bigw_d[:])
            wcols_s = consts.tile([128, BC * 16], BF16)
            nc.sync.dma_start(wcols_s[:], wcols_d[:])
            onesc_s = consts.tile([128, BC * 16], BF16)
            nc.sync.dma_start(onesc_s[:], onesc_d[:])
            oh8_s = consts.tile([16, BC * 128], BF16)
            nc.sync.dma_start(oh8_s[:], oh8_d[:])
            u2_s = consts.tile([128, 1], F32)
            nc.sync.dma_start(u2_s[:], u2_d[:])
            bv2_s = consts.tile([128, 1], F32)
            nc.sync.dma_start(bv2_s[:], bv2_d[:])
            one1_s = consts.tile([1, 128], F32)
            nc.sync.dma_start(one1_s[:], one1_d[:])

            # persistent bf16 stores: eeg residual + v, [128, 512] x 32 each
            xstore = store.tile([128, BC * NP * CH], BF16)  # 32 KB/part
            vstore = store.tile([128, BC * NP * CH], BF16)  # 32 KB/part
            # unnormalized attn rows: row 2b+h = chunk 2p+h, batch b
            attn_un = smp.tile([16, NP * CH], BF16)

            def stsl(b, p):
                return slice((b * NP + p) * CH, (b * NP + p + 1) * CH)

            # ---------------- pass A ----------------
            score_banks = []
            for p in range(NP):
                sb = pscp.tile([16, CH], F32, tag="psc", name=f"psc{p}")
                score_banks.append(sb)
                for b in range(BC):
                    xsl = xstore[:, stsl(b, p)]
                    nc.sync.dma_start(xsl, xt_d[b, p])
                    yt = ytp.tile([128, CH], BF16, tag="yt")
                    nc.sync.dma_start(yt[:], yt_d[b, p])
                    zx = pzvp.tile([128, CH], F32, tag="pzv")
                    nc.tensor.matmul(zx[:], bigg_s[:], xsl, start=True, stop=True)
                    vp = pzvp.tile([128, CH], F32, tag="pzv")
                    nc.tensor.matmul(vp[:], bigw_s[:], yt[:], start=True, stop=True)
                    # m = (z + u) * y   (full height, one DVE op)
                    m = msp.tile([128, CH], BF16, tag="ms")
                    nc.vector.scalar_tensor_tensor(
                        m[:], zx[:], u2_s[:], yt[:], op0=ALU.add, op1=ALU.mult
                    )
                    # vstore = v + bv  (scalar engine, PSUM -> SBUF bf16)
                    nc.scalar.activation(
                        vstore[:, stsl(b, p)], vp[:], AF.Identity, bias=bv2_s[:]
                    )
                    # score rows 2b / 2b+1 += w.x  then  += sum(m)
                    bsl = slice(b * 16, (b + 1) * 16)
                    nc.tensor.matmul(
                        sb[:], wcols_s[:, bsl], xsl,
                        start=(b == 0), stop=False,
                    )
                    nc.tensor.matmul(
                        sb[:], onesc_s[:, bsl], m[:],
                        start=False, stop=(b == BC - 1),
                    )

            # ---------------- softmax over T ----------------
            # per-bank max (issued as each bank completes would need loop
            # restructure; they sit here and still overlap via tile deps)
            mx4 = smp.tile([16, NP], F32)
            for p in range(NP):
                nc.vector.tensor_reduce(
                    mx4[:, p : p + 1], score_banks[p][:],
                    axis=mybir.AxisListType.X, op=ALU.max,
                )
            mx16 = smp.tile([16, 1], F32)
            nc.vector.tensor_reduce(
                mx16[:], mx4[:], axis=mybir.AxisListType.X, op=ALU.max
            )
            # partition merge via DVE 32x32 transpose: rows 2b+h -> cols
            tpin = smp.tile([32, 32], F32)
            nc.gpsimd.memset(tpin[:], 0.0)
            tpout = smp.tile([32, 32], F32)
            nc.vector.tensor_copy(tpin[0:16, 0:1], mx16[:])
            nc.vector.transpose(tpout[:], tpin[:])
            # row 0 cols 0:16 = mx16; view [1, 8, 2], pair-max -> [1, 8]
            gmax = smp.tile([1, BC], F32)
            nc.vector.tensor_reduce(
                gmax[:],
                tpout[0:1, 0:16].rearrange("p (b h) -> p b h", h=2),
                axis=mybir.AxisListType.X, op=ALU.max,
            )
            negmx = smp.tile([1, BC], F32)
            nc.vector.tensor_scalar_mul(negmx[:], gmax[:], -1.0)
            # replicate to [1, 16] (2b, 2b+1 share), transpose back to [16,1]
            tpin2 = smp.tile([32, 32], F32)
            nc.gpsimd.memset(tpin2[:], 0.0)
            r16 = tpin2[0:1, 0:16].rearrange("p (b h) -> p b h", h=2)
            nc.vector.tensor_copy(r16[:, :, 0:1], negmx[:].unsqueeze(-1))
            nc.vector.tensor_copy(r16[:, :, 1:2], negmx[:].unsqueeze(-1))
            tpout2 = smp.tile([32, 32], F32)
            nc.vector.transpose(tpout2[:], tpin2[:])
            bias16 = tpout2[0:16, 0:1]
            # exp with per-row bias; accum_out gives row sums
            z4 = smp.tile([16, NP], F32)
            for p in range(NP):
                nc.scalar.activation(
                    attn_un[:, p * CH : (p + 1) * CH], score_banks[p][:],
                    AF.Exp, bias=bias16, accum_out=z4[:, p : p + 1],
                )
            zs16 = smp.tile([16, 1], F32)
            nc.vector.tensor_reduce(
                zs16[:], z4[:], axis=mybir.AxisListType.X, op=ALU.add
            )
            tpin3 = smp.tile([32, 32], F32)
            nc.gpsimd.memset(tpin3[:], 0.0)
            nc.vector.tensor_copy(tpin3[0:16, 0:1], zs16[:])
            tpout3 = smp.tile([32, 32], F32)
            nc.vector.transpose(tpout3[:], tpin3[:])
            zrow = smp.tile([1, BC], F32)
            nc.vector.tensor_reduce(
                zrow[:],
                tpout3[0:1, 0:16].rearrange("p (b h) -> p b h", h=2),
                axis=mybir.AxisListType.X, op=ALU.add,
            )
            rzrow = smp.tile([1, BC], F32)
            nc.vector.reciprocal(rzrow[:], zrow[:])
            # broadcast rz to all 128 partitions: ones[1,128]^T (x) rz[1,8]
            rzp = pzvp.tile([128, BC], F32, tag="pzv", name="rzp")
            nc.tensor.matmul(rzp[:], one1_s[:], rzrow[:], start=True, stop=True)
            rz128 = smp.tile([128, BC], F32)
            nc.scalar.activation(rz128[:], rzp[:], AF.Identity, bias=0.0)

            # ---------------- pass B ----------------
            for b in range(BC):
                osl = slice(b * 128, (b + 1) * 128)
                for p in range(NP):
                    pa2 = pzvp.tile([128, CH], F32, tag="pzv", name=f"pa2_{b}_{p}")
                    nc.tensor.matmul(
                        pa2[:], oh8_s[:, osl],
                        attn_un[:, p * CH : (p + 1) * CH],
                        start=True, stop=True,
                    )
                    # tav = (attn * rz_b) * v
                    tav = pbp.tile([128, CH], BF16, tag="tav")
                    nc.vector.scalar_tensor_tensor(
                        tav[:], pa2[:], rz128[:, b : b + 1],
                        vstore[:, stsl(b, p)], op0=ALU.mult, op1=ALU.mult,
                    )
                    o2 = pbp.tile([128, CH], BF16, tag="o2")
                    eng = nc.gpsimd if (p % 2 == 0) else nc.vector
                    eng.tensor_add(o2[:], tav[:], xstore[:, stsl(b, p)])
                    nc.sync.dma_start(out_d[b, p], o2[:])

    nc.compile()
    return nc


def _get_nc():
    if "nc" not in _CACHE:
        _CACHE["nc"] = _build_nc()
    return _CACHE["nc"]


def _host_constants(Wq, bq, Wk, bk, Wv, bv):
    Wq64, Wk64, Wv64 = (np.asarray(a, np.float64) for a in (Wq, Wk, Wv))
    bq64, bk64 = np.asarray(bq, np.float64), np.asarray(bk, np.float64)
    G = SCALE * (Wq64.T @ Wk64)  # z[e] = sum_d G[d,e] x[d]
    w = SCALE * (Wq64.T @ bk64)
    u = SCALE * (Wk64.T @ bq64)

    BIGG = np.zeros((128, 128), np.float64)
    BIGG[0:64, 0:64] = G
    BIGG[64:128, 64:128] = G
    BIGW = np.zeros((128, 128), np.float64)
    BIGW[0:64, 0:64] = Wv64.T
    BIGW[64:128, 64:128] = Wv64.T

    WCOLS = np.zeros((128, BC * 16), np.float32)
    ONESC = np.zeros((128, BC * 16), np.float32)
    for b in range(BC):
        WCOLS[0:64, b * 16 + 2 * b] = w
        WCOLS[64:128, b * 16 + 2 * b + 1] = w
        ONESC[0:64, b * 16 + 2 * b] = 1.0
        ONESC[64:128, b * 16 + 2 * b + 1] = 1.0

    OH8 = np.zeros((16, BC * 128), np.float32)
    for b in range(BC):
        OH8[2 * b, b * 128 : b * 128 + 64] = 1.0
        OH8[2 * b + 1, b * 128 + 64 : (b + 1) * 128] = 1.0

    U2 = np.tile(u.reshape(64, 1), (2, 1)).astype(np.float32)
    BV2 = np.tile(np.asarray(bv, np.float32).reshape(64, 1), (2, 1))
    ONE1 = np.ones((1, 128), np.float32)
    return (
        BIGG.astype(NPBF16), BIGW.astype(NPBF16),
        WCOLS.astype(NPBF16), ONESC.astype(NPBF16), OH8.astype(NPBF16),
        U2, BV2, ONE1,
    )


def _pack_inputs(eeg, fnirs):
    # [T, B, D] -> [core, bi, p, (h d), t], T index = (2p+h)*CH + t
    def tr(x):
        x = np.asarray(x, np.float32).reshape(NP, 2, CH, N_CORES, BC, D)
        x = x.transpose(3, 4, 0, 1, 5, 2)  # [core, bi, p, h, d, t]
        return np.ascontiguousarray(x).reshape(N_CORES, BC, NP, 128, CH).astype(NPBF16)

    return tr(eeg), tr(fnirs)


def _prepare(eeg, fnirs, Wq, bq, Wk, bk, Wv, bv):
    BIGG, BIGW, WCOLS, ONESC, OH8, U2, BV2, ONE1 = _host_constants(
        Wq, bq, Wk, bk, Wv, bv
    )
    XT, YT = _pack_inputs(eeg, fnirs)
    return [
        {
            "XT": XT[c], "YT": YT[c],
            "BIGG": BIGG, "BIGW": BIGW, "WCOLS": WCOLS, "ONESC": ONESC,
            "OH8": OH8, "U2": U2, "BV2": BV2, "ONE1": ONE1,
        }
        for c in range(N_CORES)
    ]


def _unpack_output(outs):
    # outs: [core][bi, p, (h d), t] bf16 -> [T, B, D] fp32
    o = np.stack(outs)  # [core, bi, p, 128, t]
    o = o.reshape(N_CORES, BC, NP, 2, D, CH)
    o = o.transpose(2, 3, 5, 0, 1, 4)  # [p, h, t, core, bi, d]
    return np.ascontiguousarray(o).reshape(T, B, D).astype(np.float32)


def _run(eeg, fnirs, Wq, bq, Wk, bk, Wv, bv, **spmd_kwargs):
    from concourse.bass_utils import run_bass_kernel_spmd

    nc = _get_nc()
    in_maps = _prepare(eeg, fnirs, Wq, bq, Wk, bk, Wv, bv)
    res = run_bass_kernel_spmd(nc, in_maps, list(range(N_CORES)), **spmd_kwargs)
    return _unpack_output([res.results[c]["OUT"] for c in range(N_CORES)]), res


def kernel(eeg, fnirs, Wq, bq, Wk, bk, Wv, bv):
    return _run(eeg, fnirs, Wq, bq, Wk, bk, Wv, bv)[0]
